# revision 16
# baseline (speedup 1.0000x reference)
"""Trainium2 Bass kernel for CNNText: embedding gather + multi-width conv1d
+ bias/ReLU/max-pool + output matmul, data-parallel over batch on 8 NeuronCores.

Per core (8 batch elements):
  - Host: dedup words -> compact fp8(e4m3, x2^19) rows; ALL 8 batch elems'
    embeddings are host-pregathered into the DoubleRow pair layout (d-pairs
    per partition, K=256 per chunk).  Filters pre-transposed/scaled (x2^10)
    to fp8, grouped t-major by conv width; scales fold back out in the
    ReLU's bias operand and the bf16 output layer (max-pool commutes with
    positive scaling).
  - Device: conv = PSUM-accumulated shifted matmuls (fp8 DoubleRow, width-
    OUTER loop so weight-group DMA deadlines trail the stream), free-dim
    max reduce, relu(max+C*bias) -> bf16, and a [10,NB]-oriented output
    matmul accumulated per width with only b7's w5 column on the tail
    chain (host transposes back).
  - Startup: the first conv matmul is gated by ONE "head" transfer
    (t0 weights + b0's full embedding) split across both HWDGE queues as
    partition halves; a PE warmup burst sized to the measured DMA landing
    (~10.9us: ~0.85us issue + transfer + ~1.7us completion receipt) keeps
    the HAM clock gate ramping so conv starts warm.
"""
import os
# Fresh cores each init: hours of back-to-back NEFF runs leave the device
# in a drifted DVFS state worth ~+1-2us; a core reset at runtime init
# restores nominal clocks.  setdefault so an explicit harness choice wins.
os.environ.setdefault("NEURON_RT_RESET_CORES", "1")

import numpy as np
import ml_dtypes
from contextlib import ExitStack

import concourse.tile as tile
from concourse import bacc, mybir
from concourse.bass_utils import run_bass_kernel_spmd

# This image's antenv lacks axon_hooks; if tracing is requested via
# BASS_TRACE, bass_utils imports it. Provide a null shim so the run
# degrades to no-trace instead of crashing.
try:
    import antenv.axon_hooks  # noqa: F401
except ImportError:
    import sys as _sys
    import types as _types
    _m = _types.ModuleType("antenv.axon_hooks")
    _m.get_axon_ntff_profile_hook = lambda: None
    _m.set_axon_ntff_profile_hook = lambda h: None
    _sys.modules["antenv.axon_hooks"] = _m

P = 128
SL = 512
D = 512
B = 64
NCORES = 8
NB = B // NCORES
LAYERNUM = 100
WIDTHS = [3, 4, 5]
NT = sum(WIDTHS)          # 12 (width, offset) filter tiles
KC8 = 2                   # contraction chunks of 256 (d-pairs per partition)
NWARM = 36                # PE warmup matmuls while the head DMA lands
NCOOL = 0                 # post-conv dummy matmuls deferring the HAM drop
LPAD = 112                # filter dim padded so DR pair-dim strides are %16==0
DOUT = 10
S_E, S_K = 2.0**19, 2.0**10   # fp8 pre-scales for embedding / filters

F8 = mybir.dt.float8e4
F32 = mybir.dt.float32
BF16 = mybir.dt.bfloat16
NPF8 = ml_dtypes.float8_e4m3
NPBF16 = ml_dtypes.bfloat16

# tile t -> (group, local index); groups: head1=t0, head2=t1..2, wg4=t3..6,
# wg5=t7..11.  All groups use the t-major [p, t, j, e, f(LPAD)] layout so a
# (t, j) slice has pair-dim stride LPAD (%16==0, DR requirement).
_CACHE: dict = {}
LAST_RESULTS = None


def _build():
    nc = bacc.Bacc("TRN2", target_bir_lowering=False, debug=False,
                   enable_asserts=True, num_devices=NCORES)

    # head: per partition [w3 weights t0..t2 (t,j,e,f=112 -> 1344B) |
    # b0 emb (2048B) | b1 emb (2048B)] — everything the first ~1.3us of the
    # conv stream needs, landed as ONE split-halved transfer per queue.
    HW3 = 3 * KC8 * 2 * LPAD                     # 1344
    head_d = nc.dram_tensor("head", [P, HW3 + 2 * KC8 * SL * 2], F8,
                            kind="ExternalInput").ap()
    # w4+w5 weights combined (consumed from T0+10us on)
    wg45_d = nc.dram_tensor("wg45", [P, 9 * KC8 * 2 * LPAD], F8,
                            kind="ExternalInput").ap()
    embB_d = nc.dram_tensor("embB", [P, 6 * KC8 * SL * 2], F8,
                            kind="ExternalInput").ap()
    ol_d = nc.dram_tensor("ol", [LAYERNUM, 3 * DOUT], BF16, kind="ExternalInput").ap()
    bias_d = nc.dram_tensor("bias", [LAYERNUM, 3], F32, kind="ExternalInput").ap()
    out_d = nc.dram_tensor("out", [DOUT, NB], F32, kind="ExternalOutput").ap()
    scratch_d = nc.dram_tensor("scratch", [LAYERNUM, 1], F32, kind="ExternalOutput").ap()

    with tile.TileContext(nc) as tc:
        with ExitStack() as ctx:
            consts = ctx.enter_context(tc.tile_pool(name="consts", bufs=1))
            embp = ctx.enter_context(tc.tile_pool(name="emb", bufs=1))
            psump = ctx.enter_context(tc.tile_pool(name="psum", bufs=2, space="PSUM"))
            outp = ctx.enter_context(tc.tile_pool(name="outp", bufs=1))

            head_t = consts.tile([P, HW3 + 2 * KC8 * SL * 2], F8)
            wg45_t = consts.tile([P, 9, KC8, 2, LPAD], F8)
            embs = [embp.tile([P, KC8, SL, 2], F8, tag=f"e{b}", name=f"emb_b{b}")
                    for b in range(2, 8)]

            wt3_v = head_t[:, 0:HW3].rearrange(
                "p (t j e f) -> p t j e f", t=3, j=KC8, e=2)
            emb01_v = head_t[:, HW3:HW3 + 2 * KC8 * SL * 2].rearrange(
                "p (b j s x) -> p b j s x", b=2, j=KC8, s=SL)
            embB_v = embB_d.rearrange("p (b j x) -> p b j x", b=6, j=KC8)

            # Both HWDGE queues in consumption order of the stream.  DMA
            # completion receipts (~1-2us) serialize per queue and the
            # scalar (ACT) queue's receipts measure ~1.4us slower than
            # sync's, so the whole first ~1.3us of conv consumption rides
            # in ONE head transfer on the sync queue; every later transfer
            # has >=1.3us of modeled slack against its first consumer.
            nc.sync.dma_start(head_t[:], head_d)
            for k, b in enumerate(range(2, 8)):
                eng = nc.sync if b % 2 == 0 else nc.scalar
                eng.dma_start(
                    embs[k][:].rearrange("p j s x -> p (j s x)"),
                    embB_v[:, k].rearrange("p j x -> p (j x)"))
            nc.scalar.dma_start(wg45_t[:], wg45_d.rearrange(
                "p (t j e f) -> p t j e f", t=9, j=KC8, e=2))
            ol_t = consts.tile([LAYERNUM, 3, DOUT], BF16)
            nc.scalar.dma_start(ol_t[:], ol_d.rearrange("p (w o) -> p w o", w=3))
            bias_t = consts.tile([LAYERNUM, 3], F32)
            nc.scalar.dma_start(bias_t[:], bias_d)

            def lhsT_for(t, j):
                if t < 3:
                    return wt3_v[:, t, j, :, 0:LAYERNUM]
                return wg45_t[:, t - 3, j, :, 0:LAYERNUM]

            def rhs_for(b, j, i):
                if b < 2:
                    return emb01_v[:, b, j, i:SL, :].rearrange("p s e -> p e s")
                return embs[b - 2][:, j, i:SL, :].rearrange("p s e -> p e s")

            pooled = [outp.tile([LAYERNUM, NB], F32, tag=f"pool{wi}", name=f"pool{wi}")
                      for wi in range(3)]
            prs = [None, None, None]

            # PE warmup: throwaway matmuls during the head-DMA wait keep the
            # HAM clock gate ramping toward 8/8 before the real stream
            # starts.  GpSimd does the memset (it exits the start handshake
            # first), so warmup begins ~0.5us earlier than a DVE memset.
            warm = consts.tile([P, P], F8, name="warm")
            nc.gpsimd.memset(warm[:], 0)
            warm_ps = psump.tile([P, P], F32, tag="warm")
            for _ in range(NWARM):
                nc.tensor.matmul(warm_ps[:], lhsT=warm[:], rhs=warm[:],
                                 start=True, stop=True)

            fin2 = psump.tile([DOUT, NB], F32, tag="fin")

            def relu(wi, c0, c1):
                pr = prs[wi]
                nc.vector.tensor_scalar(pr[:, c0:c1], pooled[wi][:, c0:c1],
                                        scalar1=bias_t[:, wi:wi + 1], scalar2=0.0,
                                        op0=mybir.AluOpType.add,
                                        op1=mybir.AluOpType.max)

            t0s = [0, 3, 7]
            for wi, w in enumerate(WIDTHS):
                prs[wi] = outp.tile([LAYERNUM, NB], BF16, tag=f"pr{wi}",
                                    name=f"pr{wi}")
                for b in range(NB):
                    ps = psump.tile([LAYERNUM, SL], F32, tag=f"ps{b % 2}")
                    for i in range(w):
                        t = t0s[wi] + i
                        for j in range(KC8):
                            nc.tensor.matmul(
                                ps[:, 0:SL - i],
                                lhsT=lhsT_for(t, j),
                                rhs=rhs_for(b, j, i),
                                start=(i == 0 and j == 0),
                                stop=(i == w - 1 and j == KC8 - 1),
                                perf_mode=mybir.MatmulPerfMode.DoubleRow,
                            )
                    nc.vector.reduce_max(pooled[wi][:, b:b + 1], ps[:],
                                         axis=mybir.AxisListType.X)
                    # Deferred relus (DVE-only; the fp8->bf16 fin matmuls
                    # stay OUT of the conv stream — each DR<->normal PE
                    # mode switch costs ~0.4us of pipeline flush):
                    if wi == 1 and b == 2:
                        relu(0, 0, NB)
                    if wi == 2 and b == 2:
                        relu(1, 0, NB)
                    if wi == 2 and b == 6:
                        relu(2, 0, NB - 1)

            # Queue-warmer: a tiny DMA gated on b6's w5 pool write (~2-3us
            # before the end) keeps the sync DMA queue warm for the final
            # out transfer.
            nc.sync.dma_start(scratch_d, pooled[2][:, 6:7], single_packet=True)

            # Tail: the first three fin matmuls depend only on already-
            # relu'd columns, so the PE runs them (one mode switch) while
            # the DVE does b7's w5 reduce; only the single-column w5b
            # matmul chains after it.
            nc.tensor.matmul(fin2[:], lhsT=ol_t[:, 0, :],
                             rhs=prs[0][:], start=True, stop=False)
            nc.tensor.matmul(fin2[:], lhsT=ol_t[:, 1, :],
                             rhs=prs[1][:], start=False, stop=False)
            nc.tensor.matmul(fin2[:, 0:NB - 1], lhsT=ol_t[:, 2, :],
                             rhs=prs[2][:, 0:NB - 1], start=False, stop=False)
            relu(2, NB - 1, NB)
            nc.tensor.matmul(fin2[:, NB - 1:NB], lhsT=ol_t[:, 2, :],
                             rhs=prs[2][:, NB - 1:NB], start=False, stop=True)
            res = outp.tile([DOUT, NB], F32)
            nc.vector.tensor_copy(res[:], fin2[:])
            nc.sync.dma_start(out_d, res[:], single_packet=True)
            # The teardown (sem clears, drains) is NX-sequencer-bound, not
            # PE-array-clock-bound, so no post-conv cool-down burst: ready
            # cool matmuls would only get slotted ahead of the waiting w5b
            # matmul and delay the Tensor drain.
            if NCOOL:
                cool_ps = psump.tile([P, P], F32, tag="warm")
                for _ in range(NCOOL):
                    nc.tensor.matmul(cool_ps[:], lhsT=warm[:], rhs=warm[:],
                                     start=True, stop=True)

    nc.compile()
    return nc


def kernel(words, Embedding, outputlayer, filters_w3, bias_w3,
           filters_w4, bias_w4, filters_w5, bias_w5):
    global LAST_RESULTS
    words = np.asarray(words)
    Embedding = np.asarray(Embedding, dtype=np.float32)
    outputlayer = np.asarray(outputlayer, dtype=np.float32)
    filts = {3: np.asarray(filters_w3, dtype=np.float32),
             4: np.asarray(filters_w4, dtype=np.float32),
             5: np.asarray(filters_w5, dtype=np.float32)}
    biases = {3: np.asarray(bias_w3, dtype=np.float32),
              4: np.asarray(bias_w4, dtype=np.float32),
              5: np.asarray(bias_w5, dtype=np.float32)}

    # Dedup referenced vocab, cast only the used rows to scaled fp8, then
    # host-gather every batch element into the DoubleRow pair layout.
    uniq, inv = np.unique(words, return_inverse=True)
    table = (Embedding[uniq] * np.float32(S_E)).astype(NPF8)
    inv = inv.reshape(B, SL)

    K_all = np.stack([filts[w].reshape(LAYERNUM, w, D)[:, i, :].T
                      for w in WIDTHS for i in range(w)])    # [12, 512, 100]
    K8 = np.clip(K_all * np.float32(S_K), -240, 240).astype(NPF8)
    # DR pair layout per tile: [j, p, e, m]; groups are t-major [p, t, j, e, f]
    K8r = K8.reshape(NT, KC8, P, 2, LAYERNUM)

    def group(ts):
        g = np.zeros((P, len(ts), KC8, 2, LPAD), dtype=NPF8)
        for tl, t in enumerate(ts):
            g[:, tl, :, :, :LAYERNUM] = K8r[t].transpose(1, 0, 2, 3)
        return g

    hw3 = group([0, 1, 2]).reshape(P, 3 * KC8 * 2 * LPAD)    # [P, 1344]
    wg45 = group(list(range(3, 12))).reshape(P, 9 * KC8 * 2 * LPAD).copy()

    C = np.float32(S_E * S_K)
    ol = (outputlayer.reshape(3, LAYERNUM, DOUT).transpose(1, 0, 2) / C) \
        .astype(NPBF16).reshape(LAYERNUM, 3 * DOUT).copy()
    bias = (np.stack([biases[w] for w in WIDTHS], axis=1) * C).copy()

    in_maps = []
    for core in range(NCORES):
        ridx = inv[core * NB:(core + 1) * NB]
        g = table[ridx]                                       # [NB, SL, D]
        e = (g.reshape(NB, SL, KC8, P, 2).transpose(3, 0, 2, 1, 4)
             .reshape(P, NB, KC8 * SL * 2))
        head = np.concatenate([hw3, e[:, 0], e[:, 1]], axis=1).copy()
        embB = e[:, 2:].reshape(P, 6 * KC8 * SL * 2).copy()
        in_maps.append({"head": head, "wg45": wg45,
                        "embB": embB, "ol": ol, "bias": bias})

    nc = _CACHE.get("nc")
    if nc is None:
        nc = _CACHE["nc"] = _build()

    res = run_bass_kernel_spmd(nc, in_maps, core_ids=list(range(NCORES)))
    LAST_RESULTS = res
    return np.concatenate([res.results[i]["out"].T for i in range(NCORES)],
                          axis=0).astype(np.float32)


# revision 19
# speedup vs baseline: 1.0528x; 1.0528x over previous
"""Trainium2 Bass kernel for CNNText: embedding gather + multi-width conv1d
+ bias/ReLU/max-pool + output matmul, data-parallel over batch on 8 NeuronCores.

Per core (8 batch elements):
  - Host: dedup words -> compact fp8(e4m3, x2^19) rows; ALL 8 batch elems'
    embeddings are host-pregathered into the DoubleRow pair layout (d-pairs
    per partition, K=256 per chunk).  Filters pre-transposed/scaled (x2^10)
    to fp8, grouped t-major by conv width; scales fold back out in the
    ReLU's bias operand and the bf16 output layer (max-pool commutes with
    positive scaling).
  - Device: conv = PSUM-accumulated shifted matmuls (fp8 DoubleRow, width-
    OUTER loop so weight-group DMA deadlines trail the stream), free-dim
    max reduce, relu(max+C*bias) -> bf16, and a [10,NB]-oriented output
    matmul accumulated per width with only b7's w5 column on the tail
    chain (host transposes back).
  - Startup: the first conv matmul is gated by ONE "head" transfer
    (t0 weights + b0's full embedding) split across both HWDGE queues as
    partition halves; a PE warmup burst sized to the measured DMA landing
    (~10.9us: ~0.85us issue + transfer + ~1.7us completion receipt) keeps
    the HAM clock gate ramping so conv starts warm.
"""
import os
# Fresh cores each init: hours of back-to-back NEFF runs leave the device
# in a drifted DVFS state worth ~+1-2us; a core reset at runtime init
# restores nominal clocks.  setdefault so an explicit harness choice wins.
os.environ.setdefault("NEURON_RT_RESET_CORES", "1")

import numpy as np
import ml_dtypes
from contextlib import ExitStack

import concourse.tile as tile
from concourse import bacc, mybir
from concourse.bass_utils import run_bass_kernel_spmd

# This image's antenv lacks axon_hooks; if tracing is requested via
# BASS_TRACE, bass_utils imports it. Provide a null shim so the run
# degrades to no-trace instead of crashing.
try:
    import antenv.axon_hooks  # noqa: F401
except ImportError:
    import sys as _sys
    import types as _types
    _m = _types.ModuleType("antenv.axon_hooks")
    _m.get_axon_ntff_profile_hook = lambda: None
    _m.set_axon_ntff_profile_hook = lambda h: None
    _sys.modules["antenv.axon_hooks"] = _m

P = 128
SL = 512
D = 512
B = 64
NCORES = 8
NB = B // NCORES
LAYERNUM = 100
WIDTHS = [3, 4, 5]
NT = sum(WIDTHS)          # 12 (width, offset) filter tiles
KC8 = 2                   # contraction chunks of 256 (d-pairs per partition)
NWARM = 36                # PE warmup matmuls while the head DMA lands
NCOOL = 20                # post-conv dummy matmuls deferring the HAM drop
LPAD = 112                # filter dim padded so DR pair-dim strides are %16==0
DOUT = 10
S_E, S_K = 2.0**19, 2.0**10   # fp8 pre-scales for embedding / filters

F8 = mybir.dt.float8e4
F32 = mybir.dt.float32
BF16 = mybir.dt.bfloat16
NPF8 = ml_dtypes.float8_e4m3
NPBF16 = ml_dtypes.bfloat16

# tile t -> (group, local index); groups: head1=t0, head2=t1..2, wg4=t3..6,
# wg5=t7..11.  All groups use the t-major [p, t, j, e, f(LPAD)] layout so a
# (t, j) slice has pair-dim stride LPAD (%16==0, DR requirement).
_CACHE: dict = {}
LAST_RESULTS = None


def _build():
    nc = bacc.Bacc("TRN2", target_bir_lowering=False, debug=False,
                   enable_asserts=True, num_devices=NCORES)

    # head: per partition [w3 weights t0..t2 (t,j,e,f=112 -> 1344B) |
    # b0 emb (2048B) | b1 emb (2048B)] — everything the first ~1.3us of the
    # conv stream needs, landed as ONE split-halved transfer per queue.
    HW3 = 3 * KC8 * 2 * LPAD                     # 1344
    head_d = nc.dram_tensor("head", [P, HW3 + 2 * KC8 * SL * 2], F8,
                            kind="ExternalInput").ap()
    # w4+w5 weights combined (consumed from T0+10us on)
    wg45_d = nc.dram_tensor("wg45", [P, 9 * KC8 * 2 * LPAD], F8,
                            kind="ExternalInput").ap()
    embB_d = nc.dram_tensor("embB", [P, 6 * KC8 * SL * 2], F8,
                            kind="ExternalInput").ap()
    ol_d = nc.dram_tensor("ol", [LAYERNUM, 3 * DOUT], BF16, kind="ExternalInput").ap()
    bias_d = nc.dram_tensor("bias", [LAYERNUM, 3], F32, kind="ExternalInput").ap()
    out_d = nc.dram_tensor("out", [DOUT, NB], F32, kind="ExternalOutput").ap()
    scratch_d = nc.dram_tensor("scratch", [LAYERNUM, 1], F32, kind="ExternalOutput").ap()

    with tile.TileContext(nc) as tc:
        with ExitStack() as ctx:
            consts = ctx.enter_context(tc.tile_pool(name="consts", bufs=1))
            embp = ctx.enter_context(tc.tile_pool(name="emb", bufs=1))
            psump = ctx.enter_context(tc.tile_pool(name="psum", bufs=2, space="PSUM"))
            outp = ctx.enter_context(tc.tile_pool(name="outp", bufs=1))

            head_t = consts.tile([P, HW3 + 2 * KC8 * SL * 2], F8)
            wg45_t = consts.tile([P, 9, KC8, 2, LPAD], F8)
            embs = [embp.tile([P, KC8, SL, 2], F8, tag=f"e{b}", name=f"emb_b{b}")
                    for b in range(2, 8)]

            wt3_v = head_t[:, 0:HW3].rearrange(
                "p (t j e f) -> p t j e f", t=3, j=KC8, e=2)
            emb01_v = head_t[:, HW3:HW3 + 2 * KC8 * SL * 2].rearrange(
                "p (b j s x) -> p b j s x", b=2, j=KC8, s=SL)
            embB_v = embB_d.rearrange("p (b j x) -> p b j x", b=6, j=KC8)

            # Both HWDGE queues in consumption order of the stream.  DMA
            # completion receipts (~1-2us) serialize per queue and the
            # scalar (ACT) queue's receipts measure ~1.4us slower than
            # sync's, so the whole first ~1.3us of conv consumption rides
            # in ONE head transfer on the sync queue; every later transfer
            # has >=1.3us of modeled slack against its first consumer.
            nc.sync.dma_start(head_t[:], head_d)
            for k, b in enumerate(range(2, 8)):
                eng = nc.sync if b % 2 == 0 else nc.scalar
                eng.dma_start(
                    embs[k][:].rearrange("p j s x -> p (j s x)"),
                    embB_v[:, k].rearrange("p j x -> p (j x)"))
            nc.scalar.dma_start(wg45_t[:], wg45_d.rearrange(
                "p (t j e f) -> p t j e f", t=9, j=KC8, e=2))
            ol_t = consts.tile([LAYERNUM, 3, DOUT], BF16)
            nc.scalar.dma_start(ol_t[:], ol_d.rearrange("p (w o) -> p w o", w=3))
            bias_t = consts.tile([LAYERNUM, 3], F32)
            nc.scalar.dma_start(bias_t[:], bias_d)

            def lhsT_for(t, j):
                if t < 3:
                    return wt3_v[:, t, j, :, 0:LAYERNUM]
                return wg45_t[:, t - 3, j, :, 0:LAYERNUM]

            def rhs_for(b, j, i):
                if b < 2:
                    return emb01_v[:, b, j, i:SL, :].rearrange("p s e -> p e s")
                return embs[b - 2][:, j, i:SL, :].rearrange("p s e -> p e s")

            pooled = [outp.tile([LAYERNUM, NB], F32, tag=f"pool{wi}", name=f"pool{wi}")
                      for wi in range(3)]
            prs = [None, None, None]

            # PE warmup: throwaway matmuls during the head-DMA wait keep the
            # HAM clock gate ramping toward 8/8 before the real stream
            # starts.  GpSimd does the memset (it exits the start handshake
            # first), so warmup begins ~0.5us earlier than a DVE memset.
            warm = consts.tile([P, P], F8, name="warm")
            nc.gpsimd.memset(warm[:], 0)
            warm_ps = psump.tile([P, P], F32, tag="warm")
            for _ in range(NWARM):
                nc.tensor.matmul(warm_ps[:], lhsT=warm[:], rhs=warm[:],
                                 start=True, stop=True)

            fin2 = psump.tile([DOUT, NB], F32, tag="fin")

            def relu(wi, c0, c1):
                pr = prs[wi]
                nc.vector.tensor_scalar(pr[:, c0:c1], pooled[wi][:, c0:c1],
                                        scalar1=bias_t[:, wi:wi + 1], scalar2=0.0,
                                        op0=mybir.AluOpType.add,
                                        op1=mybir.AluOpType.max)

            t0s = [0, 3, 7]
            for wi, w in enumerate(WIDTHS):
                prs[wi] = outp.tile([LAYERNUM, NB], BF16, tag=f"pr{wi}",
                                    name=f"pr{wi}")
                for b in range(NB):
                    ps = psump.tile([LAYERNUM, SL], F32, tag=f"ps{b % 2}")
                    for i in range(w):
                        t = t0s[wi] + i
                        for j in range(KC8):
                            nc.tensor.matmul(
                                ps[:, 0:SL - i],
                                lhsT=lhsT_for(t, j),
                                rhs=rhs_for(b, j, i),
                                start=(i == 0 and j == 0),
                                stop=(i == w - 1 and j == KC8 - 1),
                                perf_mode=mybir.MatmulPerfMode.DoubleRow,
                            )
                    nc.vector.reduce_max(pooled[wi][:, b:b + 1], ps[:],
                                         axis=mybir.AxisListType.X)
                    # Deferred relus (DVE-only; the fp8->bf16 fin matmuls
                    # stay OUT of the conv stream — each DR<->normal PE
                    # mode switch costs ~0.4us of pipeline flush):
                    if wi == 1 and b == 2:
                        relu(0, 0, NB)
                    if wi == 2 and b == 2:
                        relu(1, 0, NB)
                    if wi == 2 and b == 6:
                        relu(2, 0, NB - 1)

            # Queue-warmer: a tiny DMA gated on b2's w5 pool write (~11us
            # before the end) keeps the sync DMA queue warm for the final
            # out transfer.  It must complete WELL before the out: a DMA
            # finishing within ~2us of the out issue reproducibly derails
            # the out's completion receipt (+3..5us on the teardown).
            nc.sync.dma_start(scratch_d, pooled[2][:, 2:3], single_packet=True)

            # Tail: the first three fin matmuls depend only on already-
            # relu'd columns, so the PE runs them (one mode switch) while
            # the DVE does b7's w5 reduce; only the single-column w5b
            # matmul chains after it.
            nc.tensor.matmul(fin2[:], lhsT=ol_t[:, 0, :],
                             rhs=prs[0][:], start=True, stop=False)
            nc.tensor.matmul(fin2[:], lhsT=ol_t[:, 1, :],
                             rhs=prs[1][:], start=False, stop=False)
            nc.tensor.matmul(fin2[:, 0:NB - 1], lhsT=ol_t[:, 2, :],
                             rhs=prs[2][:, 0:NB - 1], start=False, stop=False)
            relu(2, NB - 1, NB)
            nc.tensor.matmul(fin2[:, NB - 1:NB], lhsT=ol_t[:, 2, :],
                             rhs=prs[2][:, NB - 1:NB], start=False, stop=True)
            res = outp.tile([DOUT, NB], F32)
            nc.vector.tensor_copy(res[:], fin2[:])
            nc.sync.dma_start(out_d, res[:], single_packet=True)
            # A post-conv cool-down burst keeps the HAM clock gate at 8/8
            # through the out transfer's completion-receipt window — runs
            # with a dropped clock gate measure ~3x slower receipts.  The
            # burst is independent (the scheduler slots it around the tail
            # matmuls), and ends before the teardown's tensor drain needs
            # the queue.
            if NCOOL:
                cool_ps = psump.tile([P, P], F32, tag="warm")
                for _ in range(NCOOL):
                    nc.tensor.matmul(cool_ps[:], lhsT=warm[:], rhs=warm[:],
                                     start=True, stop=True)

    nc.compile()
    return nc


def kernel(words, Embedding, outputlayer, filters_w3, bias_w3,
           filters_w4, bias_w4, filters_w5, bias_w5):
    global LAST_RESULTS
    words = np.asarray(words)
    Embedding = np.asarray(Embedding, dtype=np.float32)
    outputlayer = np.asarray(outputlayer, dtype=np.float32)
    filts = {3: np.asarray(filters_w3, dtype=np.float32),
             4: np.asarray(filters_w4, dtype=np.float32),
             5: np.asarray(filters_w5, dtype=np.float32)}
    biases = {3: np.asarray(bias_w3, dtype=np.float32),
              4: np.asarray(bias_w4, dtype=np.float32),
              5: np.asarray(bias_w5, dtype=np.float32)}

    # Dedup referenced vocab, cast only the used rows to scaled fp8, then
    # host-gather every batch element into the DoubleRow pair layout.
    uniq, inv = np.unique(words, return_inverse=True)
    table = (Embedding[uniq] * np.float32(S_E)).astype(NPF8)
    inv = inv.reshape(B, SL)

    K_all = np.stack([filts[w].reshape(LAYERNUM, w, D)[:, i, :].T
                      for w in WIDTHS for i in range(w)])    # [12, 512, 100]
    K8 = np.clip(K_all * np.float32(S_K), -240, 240).astype(NPF8)
    # DR pair layout per tile: [j, p, e, m]; groups are t-major [p, t, j, e, f]
    K8r = K8.reshape(NT, KC8, P, 2, LAYERNUM)

    def group(ts):
        g = np.zeros((P, len(ts), KC8, 2, LPAD), dtype=NPF8)
        for tl, t in enumerate(ts):
            g[:, tl, :, :, :LAYERNUM] = K8r[t].transpose(1, 0, 2, 3)
        return g

    hw3 = group([0, 1, 2]).reshape(P, 3 * KC8 * 2 * LPAD)    # [P, 1344]
    wg45 = group(list(range(3, 12))).reshape(P, 9 * KC8 * 2 * LPAD).copy()

    C = np.float32(S_E * S_K)
    ol = (outputlayer.reshape(3, LAYERNUM, DOUT).transpose(1, 0, 2) / C) \
        .astype(NPBF16).reshape(LAYERNUM, 3 * DOUT).copy()
    bias = (np.stack([biases[w] for w in WIDTHS], axis=1) * C).copy()

    in_maps = []
    for core in range(NCORES):
        ridx = inv[core * NB:(core + 1) * NB]
        g = table[ridx]                                       # [NB, SL, D]
        e = (g.reshape(NB, SL, KC8, P, 2).transpose(3, 0, 2, 1, 4)
             .reshape(P, NB, KC8 * SL * 2))
        head = np.concatenate([hw3, e[:, 0], e[:, 1]], axis=1).copy()
        embB = e[:, 2:].reshape(P, 6 * KC8 * SL * 2).copy()
        in_maps.append({"head": head, "wg45": wg45,
                        "embB": embB, "ol": ol, "bias": bias})

    nc = _CACHE.get("nc")
    if nc is None:
        nc = _CACHE["nc"] = _build()

    res = run_bass_kernel_spmd(nc, in_maps, core_ids=list(range(NCORES)))
    LAST_RESULTS = res
    return np.concatenate([res.results[i]["out"].T for i in range(NCORES)],
                          axis=0).astype(np.float32)
